# revision 1
# baseline (speedup 1.0000x reference)
"""Trainium2 kernel for nn_LightningGNN: CNN node-encoder on 8 NeuronCores
(node-sharded, banded-Toeplitz conv-as-matmul), GCN/pool/classifier tail on
host. Falls back to a pure-numpy encoder if the device path fails."""

import os
import numpy as np

N_CORES = 8
T = 512
H = 64
G = 512
NB = 512                 # nodes per device block
NBLK = 13                # blocks per core
NP_CORE = NB * NBLK      # 6656 padded nodes per core


# ---------------------------------------------------------------- host math
def _conv1d_np(x, w, b, stride, pad):
    # x [n, cin, L], w [cout, cin, k]
    n, cin, L = x.shape
    cout, _, k = w.shape
    xp = np.pad(x, ((0, 0), (0, 0), (pad, pad)))
    Lo = (L + 2 * pad - k) // stride + 1
    out = np.zeros((n, cout, Lo), np.float32)
    for kk in range(k):
        sl = xp[:, :, kk:kk + stride * Lo:stride]          # [n, cin, Lo]
        out += np.einsum("ncl,oc->nol", sl, w[:, :, kk], optimize=True)
    return out + b[None, :, None]


def _encoder_numpy(x, w1, b1, w2, b2, w3, b3):
    h = x[:, None, :]
    h = np.maximum(_conv1d_np(h, w1, b1, 2, 3), 0.0)
    h = np.maximum(_conv1d_np(h, w2, b2, 2, 2), 0.0)
    h = np.maximum(_conv1d_np(h, w3, b3, 2, 2), 0.0)
    return h.mean(axis=-1).astype(np.float32)


def _gcn_tail(h, edge_index, batch, gW1, gb1, gW2, gb2, lW, lb):
    N = h.shape[0]
    src = edge_index[0].astype(np.int64)
    dst = edge_index[1].astype(np.int64)
    deg = np.bincount(dst, minlength=N).astype(np.float32) + 1.0
    dinv = 1.0 / np.sqrt(deg)
    order = np.argsort(dst, kind="stable")
    s_s, d_s = src[order], dst[order]
    seg_starts = np.flatnonzero(np.r_[True, d_s[1:] != d_s[:-1]])
    seg_ids = d_s[seg_starts]

    def layer(hin, W, b):
        hw = hin @ W
        hn = hw * dinv[:, None]
        msg = hn[s_s]                                      # [E, H] gather
        sums = np.add.reduceat(msg, seg_starts, axis=0)
        agg = np.zeros_like(hw)
        agg[seg_ids] = sums
        agg = (agg + hn) * dinv[:, None]
        return np.maximum(agg + b[None, :], 0.0)

    h1 = layer(h, gW1, gb1)
    h2 = layer(h1, gW2, gb2)
    Gn = G
    bt = batch.astype(np.int64)
    cnt = np.bincount(bt, minlength=Gn).astype(np.float32)
    bstarts = np.flatnonzero(np.r_[True, bt[1:] != bt[:-1]])
    bsums = np.add.reduceat(h2, bstarts, axis=0)
    pooled = np.zeros((Gn, h2.shape[1]), np.float32)
    pooled[bt[bstarts]] = bsums
    pooled /= np.maximum(cnt, 1.0)[:, None]
    return (pooled @ lW + lb).astype(np.float32)


# --------------------------------------------------- banded conv piece build
def _build_pieces(w1, w2, w3):
    """Return (W_pack [128, NW] f32, pieces[layer][out_block] = list of
    (src_tile, base_part, K, col_off)). Layouts:
      xT tiles  : 4 tiles [128 t, NB]   t = 128*tile + p
      y1 tiles  : 32 tiles [128=(8 t1, 16 oc), NB], t1 = 8*w + t1s
      y2 tiles  : 32 tiles [128=(4 t2, 32 oc), NB], t2 = 4*w + t2s
      y3 blocks : 32 psum [128=(2 t3, 64 oc), NB]
    """
    cols = []

    def add_piece(Wmat):  # Wmat [K, 128]
        off = 128 * len(cols)
        cols.append(np.ascontiguousarray(Wmat, np.float32))
        return off

    def w1_band(base, bexample):
        # rows: window-relative input t; cols: (t1s, oc)
        Wm = np.zeros((64, 128), np.float32)
        for t1s in range(8):
            t1 = 8 * bexample + t1s
            for oc in range(16):
                for k in range(7):
                    tin = 2 * t1 + k - 3
                    r = tin - base
                    if 0 <= tin < T and 0 <= r < 64:
                        Wm[r, t1s * 16 + oc] = w1[oc, 0, k]
        return Wm

    # conv1 piece templates
    p1 = [[] for _ in range(32)]
    tmpl_cache = {}
    for b in range(32):
        base = (16 * b - 32) if b % 2 == 0 else (16 * b - 16)
        if b == 0:
            base = 0
        Wm = w1_band(base, b)
        lo = max(base, 0)
        tile0 = lo // 128
        bp = lo % 128
        crosses = base >= 0 and (base % 128) == 96 and base + 64 <= T
        key = (b == 0, b % 2, crosses, base + 64 > T)
        if crosses:
            k2 = ("c1s", b % 2)
            if k2 not in tmpl_cache:
                tmpl_cache[k2] = (add_piece(Wm[:32]), add_piece(Wm[32:]))
            o_lo, o_hi = tmpl_cache[k2]
            p1[b].append((base // 128, 96, 32, o_lo))
            p1[b].append((base // 128 + 1, 0, 32, o_hi))
        else:
            Keff = 32 if (b == 0 or base + 64 > T) else 64
            if base + 64 > T:
                Keff = T - base
            Wcut = Wm[:Keff]
            k2 = ("c1", key)
            if k2 not in tmpl_cache:
                tmpl_cache[k2] = add_piece(Wcut)
            p1[b].append((tile0, bp, Keff, tmpl_cache[k2]))

    # conv2: out block a -> t2 in [4a, 4a+4), window t1 in [8a-2, 8a+9)
    def conv_band(wt, cin, n_ts_in, n_ts_out, tpb_out, ksz, tmin, tmax, a):
        # generic: returns list of (src_tile, base, K, Wmat)
        res = []
        t_out0 = tpb_out * a
        win_lo = 2 * t_out0 - (ksz // 2)
        win_hi = 2 * (t_out0 + tpb_out - 1) + ksz - (ksz // 2)
        # group window rows by source tile
        per_tile = {}
        for tin in range(max(win_lo, 0), min(win_hi, tmax)):
            st = tin // n_ts_in
            per_tile.setdefault(st, []).append(tin)
        for st, tins in sorted(per_tile.items()):
            t_rel0 = tins[0] % n_ts_in
            base = t_rel0 * cin
            K = len(tins) * cin
            Wm = np.zeros((K, 128), np.float32)
            for ti, tin in enumerate(tins):
                for ic in range(cin):
                    r = ti * cin + ic
                    for ts_o in range(tpb_out):
                        t_out = t_out0 + ts_o
                        k = tin - 2 * t_out + (ksz // 2)
                        if 0 <= k < ksz:
                            for oc in range(wt.shape[0]):
                                Wm[r, ts_o * wt.shape[0] + oc] = wt[oc, ic, k]
            res.append((st, base, K, Wm))
        return res

    p2 = [[] for _ in range(32)]
    c2_cache = {}
    for a in range(32):
        for st, base, K, Wm in conv_band(w2, 16, 8, 4, 4, 5, 0, 256, a):
            key = ("c2", st - a, base, K, a if a in (0, 31) and False else -1)
            h = (key, Wm.tobytes())
            hk = ("c2", st - a, base, K, hash(Wm.tobytes()))
            if hk not in c2_cache:
                c2_cache[hk] = add_piece(Wm)
            p2[a].append((st, base, K, c2_cache[hk]))

    p3 = [[] for _ in range(32)]
    for a in range(32):
        for st, base, K, Wm in conv_band(w3, 32, 4, 2, 2, 5, 0, 128, a):
            hk = ("c3", st - a, base, K, hash(Wm.tobytes()))
            if hk not in c2_cache:
                c2_cache[hk] = add_piece(Wm)
            p3[a].append((st, base, K, c2_cache[hk]))

    # fold: [128=(2 t3s, 64 oc)] -> 64 oc, mean over 64 t3 = 32 blocks * 2
    Fold = np.zeros((128, 128), np.float32)
    for t3s in range(2):
        for oc in range(64):
            Fold[t3s * 64 + oc, oc] = 1.0 / 64.0
    fold_off = add_piece(Fold)
    ident_off = add_piece(np.eye(128, dtype=np.float32))

    W_pack = np.zeros((128, 128 * len(cols)), np.float32)
    off = 0
    for c in cols:
        W_pack[:c.shape[0], off:off + 128] = c
        off += 128
    return W_pack, p1, p2, p3, fold_off, ident_off


def _encoder_sim(xc, W_pack, p1, p2, p3, fold_off, ident_off=None):
    """numpy mirror of the device dataflow, for validation."""
    nb = xc.shape[0] // NB
    enc = np.zeros((64, xc.shape[0]), np.float32)
    for i in range(nb):
        xb = xc[i * NB:(i + 1) * NB]                       # [NB, 512]
        xT = [xb[:, 128 * k:128 * (k + 1)].T.copy() for k in range(4)]
        y1, y2 = [], []
        for b in range(32):
            acc = np.zeros((128, NB), np.float32)
            for (st, bp, K, co) in p1[b]:
                Wm = W_pack[:K, co:co + 128]
                acc += Wm.T @ xT[st][bp:bp + K]
            y1.append(np.maximum(acc, 0.0))
        for a in range(32):
            acc = np.zeros((128, NB), np.float32)
            for (st, bp, K, co) in p2[a]:
                Wm = W_pack[:K, co:co + 128]
                acc += Wm.T @ y1[st][bp:bp + K]
            y2.append(np.maximum(acc, 0.0))
        encp = np.zeros((64, NB), np.float32)
        Fold = W_pack[:, fold_off:fold_off + 128]
        for a in range(32):
            acc = np.zeros((128, NB), np.float32)
            for (st, bp, K, co) in p3[a]:
                Wm = W_pack[:K, co:co + 128]
                acc += Wm.T @ y2[st][bp:bp + K]
            y3 = np.maximum(acc, 0.0)
            encp += (Fold.T @ y3)[:64]
        enc[:, i * NB:(i + 1) * NB] = encp
    return enc.T.copy()


# ------------------------------------------------------------- device path
def _encoder_on_trn(x_full, W_pack, p1, p2, p3, fold_off, ident_off):
    import concourse.bass as bass
    import concourse.mybir as mybir
    from concourse.bass_utils import run_bass_kernel_spmd
    from concourse.tile import TileContext

    dt = mybir.dt
    NW = W_pack.shape[1]
    nc = bass.Bass()
    x_in = nc.dram_tensor("xc", [NP_CORE, T], dt.float32, kind="ExternalInput")
    w_in = nc.dram_tensor("wp", [128, NW], dt.float32, kind="ExternalInput")
    enc_out = nc.dram_tensor("enc", [64, NP_CORE], dt.float32,
                             kind="ExternalOutput")

    def f32r(ap):
        return ap.bitcast(dt.float32r)

    with TileContext(nc) as tc:
        with (
            tc.tile_pool(name="wconst", bufs=1) as wpool,
            tc.tile_pool(name="xsb", bufs=2) as xpool,
            tc.tile_pool(name="xt", bufs=2) as xtpool,
            tc.tile_pool(name="y1", bufs=1) as y1pool,
            tc.tile_pool(name="y2", bufs=1) as y2pool,
            tc.tile_pool(name="ps", bufs=4, space="PSUM") as pspool,
            tc.tile_pool(name="pst", bufs=2, space="PSUM") as tppool,
            tc.tile_pool(name="pse", bufs=1, space="PSUM") as pepool,
        ):
            wt = wpool.tile([128, NW], dt.float32, tag="w")
            nc.sync.dma_start(wt[:, :], w_in[:, :])
            ident = wt  # identity lives in W_pack at ident_off

            for i in range(NBLK):
                xts = []
                for k in range(4):
                    xts.append(xtpool.tile([128, NB], dt.float32, tag=f"xt{k}", name=f"xt{k}"))
                for j in range(NB // 128):
                    xs = xpool.tile([128, T], dt.float32, tag="xs")
                    nc.sync.dma_start(
                        xs[:, :], x_in[i * NB + j * 128:i * NB + (j + 1) * 128, :])
                    for k in range(4):
                        pt = tppool.tile([128, 128], dt.float32, tag="pt")
                        nc.tensor.transpose(pt[:, :], xs[:, 128 * k:128 * (k + 1)],
                                            wt[:, ident_off:ident_off + 128])
                        nc.vector.tensor_copy(
                            xts[k][:, j * 128:(j + 1) * 128], pt[:, :])
                y1t, y2t = [], []
                for b in range(32):
                    ps = pspool.tile([128, NB], dt.float32, tag="ps")
                    for pi, (st, bp, K, co) in enumerate(p1[b]):
                        nc.tensor.matmul(
                            ps[:, :], f32r(wt[0:K, co:co + 128]),
                            f32r(xts[st][bp:bp + K, :]),
                            start=(pi == 0), stop=(pi == len(p1[b]) - 1))
                    yt = y1pool.tile([128, NB], dt.float32, tag=f"y1_{b}")
                    nc.scalar.activation(yt[:, :], ps[:, :],
                                         mybir.ActivationFunctionType.Relu)
                    y1t.append(yt)
                for a in range(32):
                    ps = pspool.tile([128, NB], dt.float32, tag="ps")
                    for pi, (st, bp, K, co) in enumerate(p2[a]):
                        nc.tensor.matmul(
                            ps[:, :], f32r(wt[0:K, co:co + 128]),
                            f32r(y1t[st][bp:bp + K, :]),
                            start=(pi == 0), stop=(pi == len(p2[a]) - 1))
                    yt = y2pool.tile([128, NB], dt.float32, tag=f"y2_{a}")
                    nc.scalar.activation(yt[:, :], ps[:, :],
                                         mybir.ActivationFunctionType.Relu)
                    y2t.append(yt)
                pe = pepool.tile([64, NB], dt.float32, tag="pe")
                for a in range(32):
                    ps = pspool.tile([128, NB], dt.float32, tag="ps")
                    for pi, (st, bp, K, co) in enumerate(p3[a]):
                        nc.tensor.matmul(
                            ps[:, :], f32r(wt[0:K, co:co + 128]),
                            f32r(y2t[st][bp:bp + K, :]),
                            start=(pi == 0), stop=(pi == len(p3[a]) - 1))
                    yt = y2pool.tile([128, NB], dt.float32, tag="y3")
                    nc.scalar.activation(yt[:, :], ps[:, :],
                                         mybir.ActivationFunctionType.Relu)
                    nc.tensor.matmul(
                        pe[:, :], f32r(wt[:, fold_off:fold_off + 64]),
                        f32r(yt[:, :]), start=(a == 0), stop=(a == 31))
                esb = xpool.tile([64, NB], dt.float32, tag="eo")
                nc.vector.tensor_copy(esb[:, :], pe[:, :])
                nc.sync.dma_start(enc_out[:, i * NB:(i + 1) * NB], esb[:, :])

    in_maps = []
    for c in range(N_CORES):
        xc = np.zeros((NP_CORE, T), np.float32)
        lo = c * (x_full.shape[0] // N_CORES)
        hi = lo + (x_full.shape[0] // N_CORES)
        xc[:hi - lo] = x_full[lo:hi]
        in_maps.append({"xc": xc, "wp": W_pack})
    res = run_bass_kernel_spmd(nc, in_maps, core_ids=list(range(N_CORES)))
    encs = [r["enc"] for r in res.results]
    per = x_full.shape[0] // N_CORES
    return np.concatenate([e[:, :per].T for e in encs], axis=0)


# ------------------------------------------------------------------- entry
def kernel(**inputs):
    x = np.asarray(inputs["x"], np.float32)
    ei = np.asarray(inputs["edge_index"])
    batch = np.asarray(inputs["batch"])
    w1 = np.asarray(inputs["w1"], np.float32)
    b1 = np.asarray(inputs["b1"], np.float32)
    w2 = np.asarray(inputs["w2"], np.float32)
    b2 = np.asarray(inputs["b2"], np.float32)
    w3 = np.asarray(inputs["w3"], np.float32)
    b3 = np.asarray(inputs["b3"], np.float32)

    W_pack, p1, p2, p3, fold_off, ident_off = _build_pieces(w1, w2, w3)
    enc = None
    if os.environ.get("KERNEL_NO_TRN") != "1":
        try:
            enc = _encoder_on_trn(x, W_pack, p1, p2, p3, fold_off, ident_off)
        except Exception as e:  # noqa: BLE001
            import traceback
            traceback.print_exc()
            enc = None
    if enc is None:
        enc = _encoder_numpy(x, w1, b1, w2, b2, w3, b3)
    # biases b1..b3 are zeros in this problem; device path ignores them.
    if np.abs(b1).max() + np.abs(b2).max() + np.abs(b3).max() > 0:
        enc = _encoder_numpy(x, w1, b1, w2, b2, w3, b3)
    return _gcn_tail(enc, ei, batch,
                     np.asarray(inputs["gW1"], np.float32),
                     np.asarray(inputs["gb1"], np.float32),
                     np.asarray(inputs["gW2"], np.float32),
                     np.asarray(inputs["gb2"], np.float32),
                     np.asarray(inputs["lW"], np.float32),
                     np.asarray(inputs["lb"], np.float32))

